# revision 2
# baseline (speedup 1.0000x reference)
"""Multi-head self-attention kernel for 8 Trainium2 NeuronCores, v3.

Problem: B=2, S=2048, D=1024, H=16 heads, head_dim=64 (all fp32).

Sharding: tensor-parallel over heads. Core c owns heads {2c, 2c+1} = feature
range [c*P, (c+1)*P) of Q/K/V and the matching P contraction rows of Wo.
Host sums the 8 full-shape partials and adds bo.

v3 keeps the whole attention inner loop ACT(exp)-bound by running the two
heads' PE work concurrently via ROW tiling (this toolchain rejects any
tile_position with col != 0, so column tiling is unavailable):
  - pass = 128 k-positions x one 512-q slice, both heads:
      scores: 2 concurrent row-tiled matmuls (64x128 tiles (0,0)/(64,0)):
        bank X [128k, 512q] = h0, bank Y = h1, one [128, 1024] PSUM tile.
      one exp per pass: ACT [128, 1024] PSUM->SBUF (pt, f32r).
      ctx: 4 row-tiled matmuls (k-halves x heads, M=65 with a ones column
        appended to V so the softmax denominators fall out for free) into
        4 separate accumulator banks (cpA/cpB = h0 k-low/k-high, cpC/cpD =
        h1), issued as 2 concurrent-pair rounds.
  - qslice tail: denominators = row 64 of cpA+cpB (resp. cpC+cpD); DVE
    reciprocal; GpSimd partition_broadcast; DVE merge (cpA+cpB)*recip into
    the persistent normalized ctxT (CX).
Batch-1 projections run at batch-0 qslice tails, outproj(0) at batch-1
tails (PSUM comes from the freed accumulator ring), outproj(1) at the end.
"""

import functools
import os
import sys

import numpy as np

for _p in ("/opt/trn_rl_repo", os.path.expanduser("~/.axon_site/_ro/trn_rl_repo")):
    if os.path.isdir(_p) and _p not in sys.path:
        sys.path.insert(0, _p)

import concourse.bass as bass
import concourse.tile as tile
from concourse import bacc
from concourse import mybir
from concourse.bass_utils import run_bass_kernel_spmd

F32 = mybir.dt.float32
F32R = mybir.dt.float32r
AF = mybir.ActivationFunctionType

P = 128          # partitions / feature slice per core
B = 2            # batch
S = 2048         # sequence length
D = 1024         # embed dim
T = B * S        # total tokens
HD = 64          # head dim
KO = D // P      # contraction subtiles for the projections
NT = 8           # t-tiles for the projections
TW = 512         # q-slice width / matmul free dim
KP = 128         # k-positions per pass
NPASS = S // KP  # 16 passes per q-slice
NQS = S // TW    # 4 q-slices per batch
N_CORES = 8
SCALE = 1.0 / np.sqrt(np.float32(HD))  # 0.125


def _build_nc(n_reps: int = 1):
    nc = bacc.Bacc(target_bir_lowering=False, debug=False, num_devices=N_CORES)

    # xt4[tt, ki, ko, t] = x[tt*TW + t, ko*P + ki]; per-partition-contiguous DMA
    xt4 = nc.declare_dram_parameter("xt4", [NT, P, KO, TW], F32, isOutput=False)
    wqT = nc.declare_dram_parameter("wqT", [P, KO, P], F32, isOutput=False)
    wkT = nc.declare_dram_parameter("wkT", [P, KO, P], F32, isOutput=False)
    wvT = nc.declare_dram_parameter("wvT", [P, KO, P], F32, isOutput=False)
    woT = nc.declare_dram_parameter("woT", [P, D], F32, isOutput=False)
    bq = nc.declare_dram_parameter("bq", [P, 1], F32, isOutput=False)
    bk = nc.declare_dram_parameter("bk", [P, 1], F32, isOutput=False)
    bv = nc.declare_dram_parameter("bv", [P, 1], F32, isOutput=False)
    out = nc.declare_dram_parameter("out", [T, D], F32, isOutput=True)

    with tile.TileContext(nc) as tc:
        from contextlib import ExitStack, nullcontext

        with ExitStack() as ctx:
            singles = ctx.enter_context(tc.tile_pool(name="singles", bufs=1))
            qkv = ctx.enter_context(tc.tile_pool(name="qkv", bufs=1))
            xpool = ctx.enter_context(tc.tile_pool(name="xpool", bufs=2))
            ptpool = ctx.enter_context(tc.tile_pool(name="ptpool", bufs=3))
            vpool = ctx.enter_context(tc.tile_pool(name="vpool", bufs=2))
            rpool = ctx.enter_context(tc.tile_pool(name="rpool", bufs=1))
            opool = ctx.enter_context(tc.tile_pool(name="opool", bufs=3))
            sppool = ctx.enter_context(
                tc.tile_pool(name="sppool", bufs=2, space="PSUM")
            )
            cpool = ctx.enter_context(tc.tile_pool(name="cpool", bufs=4, space="PSUM"))
            dpool = ctx.enter_context(tc.tile_pool(name="dpool", bufs=2, space="DRAM"))
            pools = (singles, qkv, xpool, ptpool, vpool, rpool, opool,
                     sppool, cpool, dpool, xt4, wqT, wkT, wvT, woT, bq, bk, bv, out)

            rep_loop = tc.For_i(0, n_reps, 1) if n_reps > 1 else nullcontext()
            with rep_loop:
                _kernel_body(nc, tc, pools)

    nc.finalize()
    return nc


def _kernel_body(nc, tc, pools):
    (singles, qkv, xpool, ptpool, vpool, rpool, opool,
     sppool, cpool, dpool, xt4, wqT, wkT, wvT, woT, bq, bk, bv, out) = pools

    # ---- weights / biases to SBUF ----
    w_sbs = []
    for name, wT in (("wq", wqT), ("wk", wkT), ("wv", wvT)):
        w_sb = singles.tile([P, KO, P], F32R, tag=f"{name}_sb")
        nc.sync.dma_start(out=w_sb[:], in_=wT[:].bitcast(F32R))
        w_sbs.append(w_sb)
    wo_sb = singles.tile([P, D], F32R, tag="wo_sb")
    nc.sync.dma_start(out=wo_sb[:], in_=woT[:].bitcast(F32R))
    b_sbs = []
    for name, bdram in (("bq", bq), ("bk", bk), ("bv", bv)):
        b_sb = singles.tile([P, 1], F32, tag=f"{name}_sb")
        nc.sync.dma_start(out=b_sb[:], in_=bdram[:])
        b_sbs.append(b_sb)

    # ---- persistent activations ----
    QT = qkv.tile([P, T], F32R, tag="QT")
    KT = qkv.tile([P, T], F32R, tag="KT")
    VT = qkv.tile([P, T], F32R, tag="VT")
    CX = qkv.tile([P, T], F32R, tag="CX")  # normalized ctxT, both heads

    # ---- projections: QT/KT/VT[f, t] = sum_d W[d, f] * xT[d, t] ----
    # ps_pool_fn returns a [P, TW] f32 PSUM tile (sppool early, cpool when
    # interleaved into attention so the scores double-buffer is untouched).
    def proj_tile(tt, ps_pool_fn):
        xt = xpool.tile([P, KO, TW], F32R, tag="xt", name=f"xt_{tt}")
        for q4 in range(4):
            nc.sync.dma_start(
                out=xt[:, 2 * q4 : 2 * q4 + 2],
                in_=xt4[:][tt, :, 2 * q4 : 2 * q4 + 2].bitcast(F32R),
            )
        for di, (w_sb, b_sb, dst) in enumerate(
            zip(w_sbs, b_sbs, (QT, KT, VT), strict=True)
        ):
            ps = ps_pool_fn(f"pp_{tt}_{di}")
            for ko in range(KO):
                nc.tensor.matmul(
                    ps[:],
                    w_sb[:, ko],
                    xt[:, ko],
                    start=(ko == 0),
                    stop=(ko == KO - 1),
                )
            nc.vector.tensor_scalar_add(
                dst[:, tt * TW : (tt + 1) * TW], ps[:], b_sb[:]
            )
            yield

    # ---- V' build for one batch: per head, [128, NPASS, 65] f32r where
    # partitions 0-63 hold the pass's k-low 64 positions, 64-127 the k-high,
    # and column 64 is the ones column (softmax denominator trick). ----
    def vbuild(bb: int):
        base = bb * S
        vps = []
        for h in range(2):
            vp32 = vpool.tile(
                [P, NPASS, HD + 1], F32, tag=f"vp32_{h}", name=f"vp32_{bb}_{h}"
            )
            nc.vector.memset(vp32[:, :, HD], 1.0)
            src = VT[h * HD : (h + 1) * HD, base : base + S]
            src = src.bitcast(F32).rearrange("p (j r) -> p j r", r=KP)
            for half in range(2):       # dest partition half = k-half
                for a in range(2):      # source d 32-blocks
                    for b2 in range(2):  # dest k 32-blocks within half
                        nc.vector.transpose(
                            vp32[
                                half * HD + 32 * b2 : half * HD + 32 * (b2 + 1),
                                :,
                                32 * a : 32 * (a + 1),
                            ],
                            src[
                                32 * a : 32 * (a + 1),
                                :,
                                half * HD + 32 * b2 : half * HD + 32 * (b2 + 1),
                            ],
                        )
            vp = vpool.tile(
                [P, NPASS, HD + 1], F32R, tag=f"vp_{h}", name=f"vp_{bb}_{h}"
            )
            nc.vector.tensor_copy(vp[:], vp32[:])
            vps.append(vp)
        return vps

    # ---- attention for one batch (generator; yields after each pass and
    # each qslice tail; tail_work(qs) is drained right after each tail) ----
    def batch_attn(bb: int, vps, tail_work=None):
        base = bb * S
        vp0, vp1 = vps
        for qs in range(NQS):
            q0 = base + qs * TW
            cpA = cpool.tile([P, TW], F32, tag="cb", name=f"cpA_{bb}_{qs}")
            cpB = cpool.tile([P, TW], F32, tag="cb", name=f"cpB_{bb}_{qs}")
            cpC = cpool.tile([P, TW], F32, tag="cb", name=f"cpC_{bb}_{qs}")
            cpD = cpool.tile([P, TW], F32, tag="cb", name=f"cpD_{bb}_{qs}")
            for j in range(NPASS):
                k0 = base + j * KP
                sp = sppool.tile(
                    [P, 2 * TW], F32, tag="sp", name=f"sp_{bb}_{qs}_{j}"
                )
                # scores: row-tiled pair, concurrent (tiles (0,0) / (64,0))
                nc.tensor.matmul(
                    sp[:, 0:TW],
                    KT[0:HD, k0 : k0 + KP],
                    QT[0:HD, q0 : q0 + TW],
                    start=True, stop=True,
                )
                nc.tensor.matmul(
                    sp[:, TW : 2 * TW],
                    KT[HD:P, k0 : k0 + KP],
                    QT[HD:P, q0 : q0 + TW],
                    start=True, stop=True,
                )
                # one exp per pass
                pt = ptpool.tile(
                    [P, 2 * TW], F32R, tag="pt", name=f"pt_{bb}_{qs}_{j}"
                )
                nc.scalar.activation(pt[:], sp[:], AF.Exp, scale=float(SCALE))
                # ctx + denominators: 2 concurrent-pair rounds
                st = j == 0
                sp_ = j == NPASS - 1
                nc.tensor.matmul(  # h0 k-low  -> cpA   (tile (0,0))
                    cpA[0 : HD + 1, :], vp0[0:HD, j], pt[0:HD, 0:TW],
                    start=st, stop=sp_,
                )
                nc.tensor.matmul(  # h1 k-high -> cpD   (tile (64,0))
                    cpD[0 : HD + 1, :], vp1[HD:P, j], pt[HD:P, TW : 2 * TW],
                    start=st, stop=sp_,
                )
                nc.tensor.matmul(  # h1 k-low  -> cpC   (tile (0,0))
                    cpC[0 : HD + 1, :], vp1[0:HD, j], pt[0:HD, TW : 2 * TW],
                    start=st, stop=sp_,
                )
                nc.tensor.matmul(  # h0 k-high -> cpB   (tile (64,0))
                    cpB[0 : HD + 1, :], vp0[HD:P, j], pt[HD:P, 0:TW],
                    start=st, stop=sp_,
                )
                yield
            # ---- qslice tail: denominators, reciprocal, merge into CX ----
            # All DVE work stays on partitions 0-64 (ctx outputs are forced to
            # base partition 0); h0 in free-columns [0:TW], h1 in [TW:2TW].
            srow = rpool.tile([P, 2 * TW], F32, tag="srow", name=f"srow_{bb}_{qs}")
            nc.vector.tensor_copy(srow[HD : HD + 1, 0:TW], cpB[HD : HD + 1, :])
            nc.vector.tensor_copy(srow[HD : HD + 1, TW : 2 * TW], cpD[HD : HD + 1, :])
            rs = rpool.tile([P, 2 * TW], F32, tag="rs", name=f"rs_{bb}_{qs}")
            nc.vector.tensor_add(
                out=rs[HD : HD + 1, 0:TW],
                in0=cpA[HD : HD + 1, :],
                in1=srow[HD : HD + 1, 0:TW],
            )
            nc.vector.tensor_add(
                out=rs[HD : HD + 1, TW : 2 * TW],
                in0=cpC[HD : HD + 1, :],
                in1=srow[HD : HD + 1, TW : 2 * TW],
            )
            rr = rpool.tile([P, 2 * TW], F32, tag="rr", name=f"rr_{bb}_{qs}")
            nc.vector.reciprocal(rr[HD : HD + 1, :], rs[HD : HD + 1, :])
            # broadcast the reciprocals to partitions 0-63 via a DRAM bounce
            # with a partition-step-0 read (the known-good groupnorm pattern)
            drb = dpool.tile([1, 2 * TW], F32, tag="drb", name=f"drb_{bb}_{qs}")
            nc.sync.dma_start(out=drb[:], in_=rr[HD : HD + 1, :])
            rb = rpool.tile([P, 2 * TW], F32, tag="rb", name=f"rb_{bb}_{qs}")
            nc.gpsimd.dma_start(
                out=rb[0:HD, :], in_=drb[:].to_broadcast([HD, 2 * TW])
            )
            c2a = rpool.tile([P, 2 * TW], F32, tag="c2a", name=f"c2a_{bb}_{qs}")
            nc.vector.tensor_copy(c2a[0:HD, 0:TW], cpB[0:HD, :])
            nc.vector.tensor_copy(c2a[0:HD, TW : 2 * TW], cpD[0:HD, :])
            u = rpool.tile([P, 2 * TW], F32, tag="u", name=f"u_{bb}_{qs}")
            nc.vector.tensor_add(
                out=u[0:HD, 0:TW], in0=cpA[0:HD, :], in1=c2a[0:HD, 0:TW]
            )
            nc.vector.tensor_add(
                out=u[0:HD, TW : 2 * TW],
                in0=cpC[0:HD, :],
                in1=c2a[0:HD, TW : 2 * TW],
            )
            # h0: straight into CX rows 0-63
            nc.vector.tensor_mul(
                out=CX[0:HD, q0 : q0 + TW],
                in0=u[0:HD, 0:TW],
                in1=rb[0:HD, 0:TW],
            )
            # h1: scale on partitions 0-63, then DRAM-bounce into CX rows 64-127
            th1 = rpool.tile([P, TW], F32R, tag="th1", name=f"th1_{bb}_{qs}")
            nc.vector.tensor_mul(
                out=th1[0:HD, :],
                in0=u[0:HD, TW : 2 * TW],
                in1=rb[0:HD, TW : 2 * TW],
            )
            dsc = dpool.tile([HD, TW], F32R, tag="dsc", name=f"dsc_{bb}_{qs}")
            nc.sync.dma_start(out=dsc[:], in_=th1[0:HD, :])
            nc.sync.dma_start(out=CX[HD:P, q0 : q0 + TW], in_=dsc[:])
            if tail_work is not None:
                tail_work(bb, qs)
            yield

    # ---- output projection half-steps for a range of t-chunks ----
    def outproj(bb: int, tc_lo: int, tc_hi: int):
        for tci in range(tc_lo, tc_hi):
            tg = bb * (S // P) + tci
            ot = opool.tile([P, D], F32, tag="ot")
            for half in range(2):
                ps = cpool.tile([P, TW], F32, tag="cb", name=f"op_{tg}_{half}")
                nc.tensor.matmul(
                    ps[:],
                    CX[:, tg * P : (tg + 1) * P],
                    wo_sb[:, half * TW : (half + 1) * TW],
                    start=True,
                    stop=True,
                )
                nc.vector.tensor_copy(ot[:, half * TW : (half + 1) * TW], ps[:])
                nc.sync.dma_start(
                    out=out[:][
                        tg * P : (tg + 1) * P, half * TW : (half + 1) * TW
                    ],
                    in_=ot[:, half * TW : (half + 1) * TW],
                )
                yield

    # ---- schedule ----
    def cpool_ps(name):
        return cpool.tile([P, TW], F32, tag="cb", name=name)

    def sppool_ps(name):
        return sppool.tile([P, TW], F32, tag="sp", name=name)

    # batch-0 projections upfront (scores pool is free then)
    for tt in range(NT // 2):
        for _ in proj_tile(tt, sppool_ps):
            pass

    vp0s = vbuild(0)

    # batch-1 projections dropped in at batch-0 qslice tails (PSUM from the
    # freed accumulator ring)
    def b0_tail(bb, qs):
        if qs < NT // 2:
            for _ in proj_tile(NT // 2 + qs, cpool_ps):
                pass
        if qs == NQS - 1:
            b0_tail.vp1s = vbuild(1)

    for _ in batch_attn(0, vp0s, b0_tail):
        pass

    op0 = outproj(0, 0, S // P)

    def b1_tail(bb, qs):
        for _ in range(8):
            next(op0, None)

    for _ in batch_attn(1, b0_tail.vp1s, b1_tail):
        pass
    for _ in op0:
        pass
    for _ in outproj(1, 0, S // P):
        pass


@functools.lru_cache(maxsize=8)
def _get_nc(n_reps: int = 1):
    return _build_nc(n_reps)


def _shard_inputs(x, Wq, bq, Wk, bk, Wv, bv, Wo, bo):
    x = np.asarray(x, dtype=np.float32)
    # xt4[tt, ki, ko, t] = x[tt*TW + t, ko*P + ki]
    xt4 = np.ascontiguousarray(
        x.reshape(NT, TW, KO, P).transpose(0, 3, 2, 1)
    )
    Wq = np.asarray(Wq, dtype=np.float32)
    Wk = np.asarray(Wk, dtype=np.float32)
    Wv = np.asarray(Wv, dtype=np.float32)
    Wo = np.asarray(Wo, dtype=np.float32)
    bq = np.asarray(bq, dtype=np.float32)
    bk = np.asarray(bk, dtype=np.float32)
    bv = np.asarray(bv, dtype=np.float32)

    def wtile(W, sl):
        # [ki, ko, f] = W[c*P + f, ko*P + ki]
        return np.ascontiguousarray(
            W[sl, :].reshape(P, KO, P).transpose(2, 1, 0)
        )

    in_maps = []
    for c in range(N_CORES):
        sl = slice(c * P, (c + 1) * P)
        in_maps.append(
            {
                "xt4": xt4,
                "wqT": wtile(Wq, sl),
                "wkT": wtile(Wk, sl),
                "wvT": wtile(Wv, sl),
                "woT": np.ascontiguousarray(Wo[:, sl].T),
                "bq": np.ascontiguousarray(bq[sl].reshape(P, 1)),
                "bk": np.ascontiguousarray(bk[sl].reshape(P, 1)),
                "bv": np.ascontiguousarray(bv[sl].reshape(P, 1)),
            }
        )
    return in_maps


def kernel(x, Wq, bq, Wk, bk, Wv, bv, Wo, bo, **run_kwargs):
    nc = _get_nc()
    in_maps = _shard_inputs(x, Wq, bq, Wk, bk, Wv, bv, Wo, bo)
    last_exc = None
    for _attempt in range(3):
        try:
            res = run_bass_kernel_spmd(
                nc, in_maps, core_ids=list(range(N_CORES)), **run_kwargs
            )
            break
        except Exception as exc:  # transient device errors: retry
            last_exc = exc
            import time as _time

            _time.sleep(3.0)
            try:
                import jax as _jax

                _jax.clear_caches()
                _jax.extend.backend.clear_backends()
            except Exception:
                pass
    else:
        raise last_exc
    partials = [r["out"] for r in res.results]
    acc = np.add.reduce([np.asarray(p, dtype=np.float32) for p in partials], axis=0)
    acc = acc + np.asarray(bo, dtype=np.float32)[None, :]
    if run_kwargs:
        kernel.last_results = res
    return acc.reshape(B, S, D).astype(np.float32)
